# revision 1
# baseline (speedup 1.0000x reference)
"""Trainium2 Bass kernel for the ConvFeatureExtractor problem.

Reference computation (all f32):
    matches[f, i] = sum_j kmer_params[f, kmer_idcs[i, j], j]      # (F, M)
    probs = softmax(matches / temperature, axis=1)                # over M
    pooled = freq @ probs.T                                       # (B, F)
    profile = pooled / pooled.sum(axis=1, keepdims=True)

Shapes: B=1024, M=4096 (=4^6 kmers), F=8192 filters, K=6, 4 bases.

Kernel strategy (8 NeuronCores, filter-sharded: FL = F/8 = 1024 per core):
  * matches^T = onehot(M, 24) @ params_flat^T(24, FL) as a K=24 matmul,
    where onehot one-hot-encodes kmer_idcs (built on host from the int32
    index input; it is a pure re-encoding of that input).
  * E = exp(matches/T) unnormalized (softmax denominator deferred):
    PSUM -> ACT exp -> SBUF bf16, in (M-partition, FL-free) layout.
  * U = freq @ E^T via PE bf16 matmuls accumulating over M in PSUM.
  * Z[f] = sum_i E[i, f] via DVE accumulation over M-tiles + a ones-column
    matmul for the final 128->1 partition reduction.
  * pooled = U * (1/Z) broadcast; s_part[b] = rowsum_f(pooled) per core;
    4KB AllReduce of s over the 8 cores; profile = pooled * (1/s).
Each core returns its (B, FL) f32 slice; host concatenates along F.
"""

import os

import numpy as np
import ml_dtypes

import concourse.bass as bass  # noqa: F401  (AP types come through tile/bacc)
import concourse.tile as tile
from concourse import bacc, mybir
from concourse.bass_utils import run_bass_kernel_spmd

NCORES = 8
B = 1024           # batch
M = 4096           # 4^6 kmers
F = 8192           # filters
KMER = 6           # kmer length
NBASE = 4
KK = NBASE * KMER  # 24 flattened (base, position)
FL = F // NCORES   # 1024 filters per core

MT = M // 128      # 32 contraction tiles
BT = B // 128      # 8 batch tiles
FC = 512           # psum free chunk
NFC = FL // FC     # 2

BF16 = mybir.dt.bfloat16
F32 = mybir.dt.float32
AFT = mybir.ActivationFunctionType
ALU = mybir.AluOpType

_CACHE: dict = {}


def _body(tc, freqT, onehotT, paramsT, tempr, out):
    nc = tc.nc
    with (
        tc.tile_pool(name="res", bufs=1) as res,
        tc.tile_pool(name="pm", bufs=2, space="PSUM") as pm,
        tc.tile_pool(name="pu", bufs=2, space="PSUM") as pu,
        tc.tile_pool(name="pz", bufs=2, space="PSUM") as pz,
        tc.tile_pool(name="dram", bufs=1, space="DRAM") as dram,
        tc.tile_pool(name="outp", bufs=1) as outp,
    ):
        # ---------- small inputs / constants ----------
        oh_sb = res.tile([KK, M], BF16)
        nc.sync.dma_start(oh_sb[:], onehotT[:])
        par_sb = res.tile([KK, FL], BF16)
        nc.sync.dma_start(par_sb[:], paramsT[:])
        t_sb = res.tile([128, 1], F32)       # T replicated on host to (128,1)
        nc.sync.dma_start(t_sb[:], tempr[:])
        invt_bc = res.tile([128, 1], F32)    # per-partition 1/T activation scale
        nc.vector.reciprocal(invt_bc[:], t_sb[:])
        ones_bf = res.tile([128, 128], BF16)  # lhsT: partition-sum + broadcast
        nc.vector.memset(ones_bf[:], 1.0)

        # ---------- stream in freq^T (M, B) as 32 k-tiles ----------
        freq_sb = res.tile([128, MT * B], BF16)
        for k in range(MT):
            nc.sync.dma_start(freq_sb[:, k * B:(k + 1) * B],
                              freqT[k * 128:(k + 1) * 128, :])

        # ---------- matches^T -> E = exp(matches/T); Z accumulation ----------
        E_sb = res.tile([128, MT * FL], BF16)
        zacc = res.tile([128, FL], F32)
        nc.vector.memset(zacc[:], 0.0)
        for k in range(MT):
            for fc in range(NFC):
                pm_t = pm.tile([128, FC], F32, tag="pm")
                nc.tensor.matmul(pm_t[:],
                                 lhsT=oh_sb[:, k * 128:(k + 1) * 128],
                                 rhs=par_sb[:, fc * FC:(fc + 1) * FC],
                                 start=True, stop=True)
                nc.scalar.activation(
                    E_sb[:, k * FL + fc * FC: k * FL + (fc + 1) * FC],
                    pm_t[:], AFT.Exp, scale=invt_bc[:])
            nc.vector.tensor_add(zacc[:], zacc[:], E_sb[:, k * FL:(k + 1) * FL])

        stage = os.environ.get("KERNEL_STAGE", "")
        if stage == "1":
            # bisect: write exp(matches/T) tiles for batch-tile-shaped slices
            for b in range(BT):
                prof = outp.tile([128, FL], F32, tag="prof")
                nc.scalar.copy(prof[:], E_sb[:, b * FL:(b + 1) * FL])
                nc.sync.dma_start(out[b * 128:(b + 1) * 128, :], prof[:])
            return

        U_sb = res.tile([128, BT * FL], F32)
        s_col = res.tile([128, BT], F32)
        invz_bc = res.tile([128, FL], F32)

        zacc_bf = res.tile([128, FL], BF16)

        def z_finish():
            # ones(128,128).T @ zacc_bf = column sums broadcast to every
            # partition, as a standard-shape bf16 matmul per chunk
            nc.scalar.copy(zacc_bf[:], zacc[:])
            for fc in range(NFC):
                zbc_ps = pz.tile([128, FC], F32, tag="pz", name=f"zbc{fc}")
                nc.tensor.matmul(zbc_ps[:], lhsT=ones_bf[:],
                                 rhs=zacc_bf[:, fc * FC:(fc + 1) * FC],
                                 start=True, stop=True)
                nc.scalar.copy(zacc[:, fc * FC:(fc + 1) * FC], zbc_ps[:])
                nc.vector.reciprocal(invz_bc[:, fc * FC:(fc + 1) * FC],
                                     zacc[:, fc * FC:(fc + 1) * FC])

        # ---------- U = freq @ E^T per batch tile; scale by 1/Z; rowsums ----
        for b in range(BT):
            pu0 = pu.tile([128, FC], F32, tag="pu0")
            pu1 = pu.tile([128, FC], F32, tag="pu1")
            if os.environ.get("KERNEL_INTERLEAVE"):
                for k in range(MT):
                    lw = freq_sb[:, k * B + b * 128: k * B + (b + 1) * 128]
                    nc.tensor.matmul(pu0[:], lhsT=lw,
                                     rhs=E_sb[:, k * FL: k * FL + FC],
                                     start=(k == 0), stop=(k == MT - 1))
                    nc.tensor.matmul(pu1[:], lhsT=lw,
                                     rhs=E_sb[:, k * FL + FC: (k + 1) * FL],
                                     start=(k == 0), stop=(k == MT - 1))
            else:
                for fc, put in ((0, pu0), (1, pu1)):
                    for k in range(MT):
                        lw = freq_sb[:, k * B + b * 128: k * B + (b + 1) * 128]
                        nc.tensor.matmul(put[:], lhsT=lw,
                                         rhs=E_sb[:, k * FL + fc * FC:
                                                  k * FL + (fc + 1) * FC],
                                         start=(k == 0), stop=(k == MT - 1))
            nc.scalar.copy(U_sb[:, b * FL: b * FL + FC], pu0[:])
            nc.scalar.copy(U_sb[:, b * FL + FC: (b + 1) * FL], pu1[:])
            if stage == "2":
                nc.sync.dma_start(out[b * 128:(b + 1) * 128, :],
                                  U_sb[:, b * FL:(b + 1) * FL])
                continue
            if b == 0:
                # emitted here so PE's in-order stream hits these tiny f32
                # matmuls right when zacc's DVE chain completes
                z_finish()
            nc.vector.tensor_mul(U_sb[:, b * FL:(b + 1) * FL],
                                 U_sb[:, b * FL:(b + 1) * FL], invz_bc[:])
            nc.vector.reduce_sum(s_col[:, b:b + 1],
                                 U_sb[:, b * FL:(b + 1) * FL],
                                 axis=mybir.AxisListType.X)

        if stage == "2":
            return
        if stage == "3":
            for b in range(BT):
                nc.sync.dma_start(out[b * 128:(b + 1) * 128, :],
                                  U_sb[:, b * FL:(b + 1) * FL])
            return

        # ---------- AllReduce of per-core rowsums (4KB) ----------
        s_sum = res.tile([128, BT], F32)
        if os.environ.get("KERNEL_NO_COLLECTIVE"):
            nc.vector.tensor_scalar_mul(s_sum[:], s_col[:], float(NCORES))
        else:
            s_in = dram.tile([128, BT], F32)
            s_out = dram.tile([128, BT], F32, addr_space="Shared")
            nc.sync.dma_start(s_in[:], s_col[:])
            nc.gpsimd.collective_compute(
                "AllReduce", ALU.add,
                replica_groups=[list(range(NCORES))],
                ins=[s_in.opt()], outs=[s_out.opt()])
            nc.sync.dma_start(s_sum[:], s_out[:])
        rinv = res.tile([128, BT], F32)
        nc.vector.reciprocal(rinv[:], s_sum[:])

        # ---------- profile = pooled * (1/s); write out ----------
        for b in range(BT):
            prof = outp.tile([128, FL], F32, tag="prof")
            nc.vector.tensor_scalar_mul(prof[:], U_sb[:, b * FL:(b + 1) * FL],
                                        rinv[:, b:b + 1])
            nc.sync.dma_start(out[b * 128:(b + 1) * 128, :], prof[:])


def _build_bass():
    nc = bacc.Bacc("TRN2", target_bir_lowering=False, debug=False,
                   num_devices=NCORES)
    freqT = nc.dram_tensor("freqT", [M, B], BF16, kind="ExternalInput").ap()
    onehotT = nc.dram_tensor("onehotT", [KK, M], BF16, kind="ExternalInput").ap()
    paramsT = nc.dram_tensor("paramsT", [KK, FL], BF16, kind="ExternalInput").ap()
    tempr = nc.dram_tensor("tempr", [128, 1], F32, kind="ExternalInput").ap()
    out = nc.dram_tensor("out", [B, FL], F32, kind="ExternalOutput").ap()

    with tile.TileContext(nc) as tc:
        _body(tc, freqT, onehotT, paramsT, tempr, out)
    nc.compile()
    return nc


def _get_nc():
    if "nc" not in _CACHE:
        _CACHE["nc"] = _build_bass()
    return _CACHE["nc"]


def _prepare_in_maps(freq, kmer_params, temperature, kmer_idcs):
    freq = np.asarray(freq, dtype=np.float32)            # (B, M)
    kp = np.asarray(kmer_params, dtype=np.float32)       # (F, 4, K)
    temp = np.asarray(temperature, dtype=np.float32).reshape(-1)[:1]
    idcs = np.asarray(kmer_idcs).astype(np.int64)        # (M, K)

    assert freq.shape == (B, M) and kp.shape == (F, NBASE, KMER)
    assert idcs.shape == (M, KMER)

    # one-hot re-encoding of the index input: onehot[i, c*K + j] = 1 iff
    # kmer_idcs[i, j] == c   (params_flat[f, c*K + j] = kmer_params[f, c, j])
    onehot = np.zeros((M, NBASE, KMER), dtype=np.float32)
    onehot[np.arange(M)[:, None], idcs, np.arange(KMER)[None, :]] = 1.0
    onehotT = np.ascontiguousarray(
        onehot.reshape(M, KK).T).astype(ml_dtypes.bfloat16)

    params_flat = kp.reshape(F, KK)
    freqT = np.ascontiguousarray(freq.T).astype(ml_dtypes.bfloat16)
    tempr = np.ascontiguousarray(np.broadcast_to(temp.reshape(1, 1), (128, 1)))

    in_maps = []
    for c in range(NCORES):
        paramsT_c = np.ascontiguousarray(
            params_flat[c * FL:(c + 1) * FL].T).astype(ml_dtypes.bfloat16)
        in_maps.append({
            "freqT": freqT,
            "onehotT": onehotT,
            "paramsT": paramsT_c,
            "tempr": tempr,
        })
    return in_maps


def _run(in_maps, trace=False):
    nc = _get_nc()
    return run_bass_kernel_spmd(nc, in_maps, list(range(NCORES)), trace=trace)


def kernel(freq, kmer_params, temperature, kmer_idcs):
    in_maps = _prepare_in_maps(freq, kmer_params, temperature, kmer_idcs)
    res = _run(in_maps,
               trace=os.environ.get("KERNEL_TRACE", "") not in ("", "0"))
    _CACHE["last_result"] = res
    return np.concatenate(
        [np.asarray(res.results[c]["out"], dtype=np.float32)
         for c in range(NCORES)], axis=1)

